# revision 31
# baseline (speedup 1.0000x reference)
"""Bayesian multi-task MLP (moe_routing) — Trainium2 Bass/Tile kernel.

Reference computation (per forward):
    w   = mu + exp(ls) * eps                    (Bayesian reparameterization)
    h   = relu(x @ w0.T + b0)                   [4096, 2048]
    h   = relu(h @ wi.T + bi)   for i in 0..2   (2048 -> 2048)
    out = (h @ hw[t].T + hb[t]) routed per-sample by task id   [4096, 10]

Distribution: pure data-parallel over the batch — each of the 8 cores gets
512 rows of x/task and a full replica of the (device-prepared) Bayesian
params.  No collectives.

Host-side prep is layout/dtype only:
  * weight factors transposed to [in, out] so the PE consumes them without
    on-chip transposes (activations stay transposed [hid, batch] throughout);
  * mu / x cast to fp16; ls shipped as (ls + 6) in fp8e4m3 (centered near 0
    where fp8 has precision); eps shipped as fp8e4m3; ls and eps interleaved
    row-wise into one tensor so each DMA line is 2 KB.

On-device weight prep per 4-k-tile group:
  ACT   t = exp(ls' - 6)        one op per group, bias AP folds the shift
  DVE   n = t * eps8            tensor_tensor, 1x (fp8 operand)
  DVE   w = n + mu              all-fp16 flat-view tensor_tensor -> 2x mode
  PE    8 accumulating matmuls  (stationary = weight tile, moving = hT batch)

Trace-driven design notes (measured on the 8-core SPMD run):
  * In steady state the PE issues 512-row matmuls back-to-back at the full
    2.4 GHz warm clock (216 ns inter-start, p25=p75); the kernel is at the
    fp16 PE roofline there.  Remaining span = ~197us of matmul issues
    + ~14us fixed startup (framework preamble + first weight chain)
    + ~5us tail (routing select + output DMA + final barrier).
  * scalar_tensor_tensor always runs the DVE at 1 elem/lane/cycle; the
    mult(1x fp8) + flat-view add(2x) pair hits the same 1.5 cyc/elem floor
    with one fewer op per tile.  3D strided views defeat the DVE 2x mode —
    fp16 operands must be flat 1D-free APs.
  * A dummy 1-elem activation precedes all DMA issues so the exp-LUT
    ACT_TABLE_LOAD's table fetch doesn't queue behind weight DMAs (-5us).
  * x DMAs ride the GpSimd DGE queue in parallel with the Sync queue's
    weight-stream issues.  (Routing the big eps streams there regressed
    30us — the GpSimd DGE handles large transfers poorly; weight DMAs
    stay on Sync.)
  * 4 head psum chains share one PSUM bank; one masked-mult + reduce +
    one output DMA finish the routing select in ~5us.
  * Tried and rejected: GPSIMD fp8->fp16 eps upconvert (CAST measures
    ~3.9 cyc/elem there), KG=8 groups (starves the 2-buf pipeline),
    mixed fp16/fp8 eps (+7MB DMA, 3 issues/group saturate the Sync DGE),
    deferred relu emission, deeper DMA lookahead (all within-noise worse).
"""

import numpy as np

import concourse.bacc as bacc
import concourse.mybir as mybir
from concourse.bass_utils import run_bass_kernel_spmd
from concourse.tile import TileContext

NCORES = 8
B, IN, H, OUT, T, NL = 4096, 1024, 2048, 10, 10, 3
BC = B // NCORES           # batch rows per core = 512
TO = T * OUT               # flattened head outputs = 100
KT_IN = IN // 128          # k-tiles in layer 0 = 8
KT_H = H // 128            # k-tiles in hidden layers = 16
KG = 4                     # k-tiles per grouped DMA / elementwise op
OT_HALF = 8                # out-tiles per half (8 PSUM banks)

F16 = mybir.dt.float16
F32 = mybir.dt.float32
F8 = mybir.dt.float8e4
ALU = mybir.AluOpType
ACTF = mybir.ActivationFunctionType


def build_nc():
    # Bacc (not raw Bass): its compile() pass legalizes multi-wait sync_infos
    # into EventSemaphore instructions (TRN2 allows 1 wait per instruction).
    nc = bacc.Bacc(trn_type="TRN2")

    # ---- per-core DRAM I/O ----
    xT = nc.dram_tensor("xT", [128, KT_IN, BC], F16, kind="ExternalInput")
    muT0 = nc.dram_tensor("muT0", [IN, H], F16, kind="ExternalInput")
    # ls/eps interleaved: [k, half, {ls,eps}, 1024] -> 2KB contiguous lines
    leT0 = nc.dram_tensor("leT0", [IN, 2, 2, 1024], F8, kind="ExternalInput")
    muT = nc.dram_tensor("muT", [NL, H, H], F16, kind="ExternalInput")
    leT = nc.dram_tensor("leT", [NL, H, 2, 2, 1024], F8, kind="ExternalInput")
    # head weights pre-tiled on host to [128, k, to] (contiguous per partition)
    muhT = nc.dram_tensor("muhT", [128, KT_H, TO], F16, kind="ExternalInput")
    lshT = nc.dram_tensor("lshT", [128, KT_H, TO], F16, kind="ExternalInput")
    epshT = nc.dram_tensor("epshT", [128, KT_H, TO], F16, kind="ExternalInput")
    # biases for the 4 dense layers, pre-tiled [128, layer, otile], fp32
    mub = nc.dram_tensor("mub", [128, NL + 1, KT_H], F32, kind="ExternalInput")
    lsb = nc.dram_tensor("lsb", [128, NL + 1, KT_H], F32, kind="ExternalInput")
    epsb = nc.dram_tensor("epsb", [128, NL + 1, KT_H], F32, kind="ExternalInput")
    muhb = nc.dram_tensor("muhb", [1, TO], F32, kind="ExternalInput")
    lshb = nc.dram_tensor("lshb", [1, TO], F32, kind="ExternalInput")
    epshb = nc.dram_tensor("epshb", [1, TO], F32, kind="ExternalInput")
    taskf = nc.dram_tensor("taskf", [128, BC // 128], F32, kind="ExternalInput")
    out = nc.dram_tensor("out", [BC, OUT], F32, kind="ExternalOutput")

    with TileContext(nc) as tc:
        with (
            tc.tile_pool(name="const", bufs=1) as cpool,
            tc.tile_pool(name="wstream", bufs=4) as wpool,
            tc.tile_pool(name="hbuf", bufs=3) as hpool,
            tc.tile_pool(name="sel", bufs=2) as spool,
            tc.tile_pool(name="psum", bufs=8, space="PSUM") as ppool,
        ):
            # per-partition -6.0 bias: folds the host's +6 fp8 shift back out
            # inside the exp, so the noise multiply needs no scalar factor
            neg6 = cpool.tile([128, 1], F32, name="neg6")
            nc.vector.memset(neg6, -6.0)
            # dummy activation before any DMA is issued: the compiler inserts
            # ACT_TABLE_LOAD before the first Scalar activation, and its table
            # fetch must not queue behind the big weight DMAs (~5us startup)
            warm = cpool.tile([1, 1], F32, name="warm")
            warm2 = cpool.tile([1, 1], F32, name="warm2")
            nc.vector.memset(warm, 0.0)
            nc.scalar.activation(out=warm2, in_=warm, func=ACTF.Exp)

            # ---- dense-layer biases b = mu + exp(ls)*eps  (tiny, fp32) ----
            # Built lazily so the first weight-group DMAs lead the DMA queue;
            # the bias is only consumed at the first relu, ~50us in.
            _bias_cache = []

            def get_bias():
                if _bias_cache:
                    return _bias_cache[0]
                bias_mu = cpool.tile([128, NL + 1, KT_H], F32, name="bias_mu")
                bias_ls = cpool.tile([128, NL + 1, KT_H], F32, name="bias_ls")
                bias_eps = cpool.tile([128, NL + 1, KT_H], F32, name="bias_eps")
                nc.sync.dma_start(out=bias_mu, in_=mub.ap())
                nc.sync.dma_start(out=bias_ls, in_=lsb.ap())
                nc.sync.dma_start(out=bias_eps, in_=epsb.ap())
                bias = cpool.tile([128, NL + 1, KT_H], F32, name="bias")
                nc.scalar.activation(out=bias, in_=bias_ls, func=ACTF.Exp)
                nc.vector.tensor_mul(bias, bias, bias_eps)
                nc.vector.tensor_add(bias, bias, bias_mu)
                _bias_cache.append(bias)
                return bias

            # ---- layer 0 input: xT resident in SBUF (graduated chunks so the
            # first matmul only waits on a 128KB slice; chunk 0 is issued
            # first, the rest after the first weight group's DMAs) ----
            # x DMAs go through the GpSimd DGE queue (~25ns issue) so they
            # never contend with the weight-stream issues on the Sync queue
            hT_x = hpool.tile([128, KT_IN, BC], F16, tag="hT")
            nc.gpsimd.dma_start(out=hT_x[:, 0:1, :], in_=xT.ap()[:, 0:1, :])

            _x_sched = {1: (1, 1), 2: (2, 2), 3: (4, 4)}

            def issue_x_chunk(gi):
                if gi in _x_sched:
                    xk, xc = _x_sched[gi]
                    nc.gpsimd.dma_start(
                        out=hT_x[:, xk:xk + xc, :], in_=xT.ap()[:, xk:xk + xc, :]
                    )

            def ff_layer(hT_in, kt, mu_ap, le_ap, bias_l, first=False,
                         fill=False, after_first_group=None):
                """hT_out[out, b] = relu(w @ hT_in + b); w = mu + exp(ls)*eps.

                mu_ap: [kt*128, H] fp16 transposed view.
                le_ap: [kt*128, 2, 2, 1024] fp8 view; [., h, 0, :] is ls+6 for
                column half h, [., h, 1, :] is eps.
                """
                hT_out = hpool.tile([128, KT_H, BC], F16, tag="hT", name="hT_out")
                for half in range(2):
                    psums = []
                    for o8 in range(OT_HALF):
                        ps = ppool.tile([128, BC], F32, tag="mm", name="ps")
                        psums.append(ps)
                    ocols = slice(half * 1024, (half + 1) * 1024)
                    # The weight stream runs at ~290GB/s against ~350GB/s DMA
                    # capacity, so the prefetch buffer only fills slowly; for
                    # the first ~60us (L0 + first half of L1) use small groups
                    # to keep per-group transfer latency off the PE's critical
                    # path.  Steady state uses KG-sized groups.
                    if first and half == 0:
                        gsizes = [1, 1, 2]
                        while sum(gsizes) < kt:
                            gsizes.append(min(KG, kt - sum(gsizes)))
                    else:
                        gsizes = [KG] * (kt // KG)
                    k0 = 0
                    for gi, gs in enumerate(gsizes):
                        grows = slice(k0 * 128, (k0 + gs) * 128)
                        mu_g = wpool.tile([128, KG, 1024], F16, tag="mu", name="mu_g")
                        le_g = wpool.tile([128, KG, 2048], F8, tag="le", name="le_g")
                        if first or fill:
                            # fill region: split the group's ls and eps into
                            # two sub-DMAs — the exp only waits on the ls
                            # half, halving the prep chain's DMA latency
                            # while the prefetch buffer is still filling
                            nc.sync.dma_start(
                                out=le_g[:, :gs, 0:1024],
                                in_=le_ap[grows, half, 0].rearrange(
                                    "(g p) o -> p g o", p=128
                                ),
                            )
                            nc.sync.dma_start(
                                out=le_g[:, :gs, 1024:2048],
                                in_=le_ap[grows, half, 1].rearrange(
                                    "(g p) o -> p g o", p=128
                                ),
                            )
                        else:
                            nc.sync.dma_start(
                                out=le_g[:, :gs, :],
                                in_=le_ap[grows, half].rearrange(
                                    "(g p) t o -> p g (t o)", p=128
                                ),
                            )
                        nc.sync.dma_start(
                            out=mu_g[:, :gs, :],
                            in_=mu_ap[grows, ocols].rearrange(
                                "(g p) o -> p g o", p=128
                            ),
                        )
                        if first and half == 0 and after_first_group:
                            after_first_group(gi)
                        # group elementwise: one op covers gs k-tiles.  All
                        # fp16 operands use flat 1D-free views ([128, gs*1024])
                        # so the DVE 2x perf mode engages on the add.
                        t_g = wpool.tile([128, KG, 1024], F16, tag="t", name="t_g")
                        w_g = wpool.tile([128, KG, 1024], F16, tag="w", name="w_g")
                        t_fl = t_g.rearrange("p g o -> p (g o)")[:, :gs * 1024]
                        w_fl = w_g.rearrange("p g o -> p (g o)")[:, :gs * 1024]
                        mu_fl = mu_g.rearrange("p g o -> p (g o)")[:, :gs * 1024]
                        nc.scalar.activation(
                            out=t_g[:, :gs, :], in_=le_g[:, :gs, 0:1024],
                            func=ACTF.Exp, bias=neg6,
                        )
                        nc.vector.tensor_tensor(
                            t_g[:, :gs, :], t_g[:, :gs, :],
                            le_g[:, :gs, 1024:2048], ALU.mult,
                        )
                        nc.vector.tensor_add(w_fl, t_fl, mu_fl)
                        for ks in range(gs):
                            k = k0 + ks
                            for o8 in range(OT_HALF):
                                nc.tensor.matmul(
                                    psums[o8],
                                    lhsT=w_g[:, ks, o8 * 128:(o8 + 1) * 128],
                                    rhs=hT_in[:, k, :],
                                    start=(k == 0),
                                    stop=(k == kt - 1),
                                )
                        k0 += gs
                    for o8 in range(OT_HALF):
                        o = half * OT_HALF + o8
                        nc.scalar.activation(
                            out=hT_out[:, o, :],
                            in_=psums[o8],
                            func=ACTF.Relu,
                            bias=get_bias()[:, bias_l, o:o + 1],
                        )
                return hT_out

            cur = ff_layer(hT_x, KT_IN, muT0.ap(), leT0.ap(), 0, first=True,
                           after_first_group=issue_x_chunk)
            for l in range(NL - 1):
                cur = ff_layer(cur, KT_H, muT.ap()[l], leT.ap()[l], l + 1,
                               fill=(l == 0))

            # ---- tail constants (issued before the last layer so their DMAs
            # land while its GEMMs run) ----
            hb_mu = cpool.tile([1, TO], F32)
            hb_ls = cpool.tile([1, TO], F32)
            hb_eps = cpool.tile([1, TO], F32)
            nc.sync.dma_start(out=hb_mu, in_=muhb.ap())
            nc.sync.dma_start(out=hb_ls, in_=lshb.ap())
            nc.sync.dma_start(out=hb_eps, in_=epshb.ap())
            hb_f = cpool.tile([1, TO], F32)
            nc.scalar.activation(out=hb_f, in_=hb_ls, func=ACTF.Exp)
            nc.vector.tensor_mul(hb_f, hb_f, hb_eps)
            nc.vector.tensor_add(hb_f, hb_f, hb_mu)
            hb16 = cpool.tile([1, TO], F16)
            nc.vector.tensor_copy(out=hb16, in_=hb_f)
            ones1 = cpool.tile([1, 128], F16)
            nc.vector.memset(ones1, 1.0)

            # head weights: whT[p, k, to] = mu + exp(ls)*eps  (fp16, ~1.2MB)
            wh_mu = cpool.tile([128, KT_H, TO], F16)
            wh_ls = cpool.tile([128, KT_H, TO], F16)
            wh_eps = cpool.tile([128, KT_H, TO], F16)
            nc.sync.dma_start(out=wh_mu, in_=muhT.ap())
            nc.sync.dma_start(out=wh_ls, in_=lshT.ap())
            nc.sync.dma_start(out=wh_eps, in_=epshT.ap())
            whT = cpool.tile([128, KT_H, TO], F16)
            nc.scalar.activation(out=whT, in_=wh_ls, func=ACTF.Exp)
            nc.vector.tensor_mul(whT, whT, wh_eps)
            nc.vector.tensor_add(whT, whT, wh_mu)

            taskt = cpool.tile([128, BC // 128], F32)
            nc.sync.dma_start(out=taskt, in_=taskf.ap())
            iota10 = cpool.tile([128, T], mybir.dt.int32)
            nc.gpsimd.iota(iota10, [[1, T]], base=0, channel_multiplier=0)
            iota10f = cpool.tile([128, T], F32)
            nc.vector.tensor_copy(out=iota10f, in_=iota10)
            # one-hot rows for all 4 m-tiles, built before the last layer
            MT = BC // 128
            onehot4 = cpool.tile([128, MT, T], F32, name="onehot4")
            for m in range(MT):
                nc.vector.tensor_single_scalar(
                    out=onehot4[:, m, :], in_=iota10f,
                    scalar=taskt[:, m:m + 1], op=ALU.is_equal,
                )

            # ---- last hidden layer ----
            cur = ff_layer(cur, KT_H, muT.ap()[NL - 1], leT.ap()[NL - 1], NL)

            # ---- heads + routing select: all 4 m-tiles in one PSUM bank ----
            ps = ppool.tile([128, MT * TO], F32, tag="mm", name="ps_head")
            for m in range(MT):
                cols = slice(m * TO, (m + 1) * TO)
                for k in range(KT_H):
                    nc.tensor.matmul(
                        ps[:, cols],
                        lhsT=cur[:, k, m * 128:(m + 1) * 128],
                        rhs=whT[:, k, :],
                        start=(k == 0),
                        stop=False,
                    )
                nc.tensor.matmul(
                    ps[:, cols], lhsT=ones1[:1, :], rhs=hb16[:1, :],
                    start=False, stop=True,
                )
            masked = spool.tile([128, MT, OUT, T], F32, name="masked")
            ps_v = ps.rearrange("p (m t o) -> p m o t", m=MT, t=T)
            oh_v = onehot4.unsqueeze(2).broadcast_to([128, MT, OUT, T])
            nc.vector.tensor_tensor(masked, ps_v, oh_v, ALU.mult)
            outm = spool.tile([128, MT, OUT], F32, name="outm")
            nc.vector.tensor_reduce(
                out=outm, in_=masked, axis=mybir.AxisListType.X, op=ALU.add
            )
            nc.sync.dma_start(
                out=out.ap().rearrange("(m p) o -> p m o", p=128), in_=outm
            )

    nc.finalize()
    return nc


_CACHE = {}


def _prep_host(inputs):
    """Layout/dtype prep + batch sharding. Returns list of per-core in_maps."""
    import ml_dtypes

    f16 = np.float16
    f8 = ml_dtypes.float8_e4m3fn

    def bias_tile(b0, b):  # [4, H] -> [128, 4, 16]
        arr = np.concatenate([b0[None], b], 0).astype(np.float32)
        return np.ascontiguousarray(arr.reshape(NL + 1, KT_H, 128).transpose(2, 0, 1))

    def head_tile(a):  # [T, OUT, H] -> headT [H, TO] -> [128, 16, TO]
        aT = a.reshape(TO, H).astype(f16).T
        return np.ascontiguousarray(aT.reshape(KT_H, 128, TO).transpose(1, 0, 2))

    def le_tile(ls, eps):
        # [K, H] x2 -> [K, half, {ls,eps}, 1024] fp8, ls pre-shifted by +6
        K = ls.shape[0]
        le = np.empty((K, 2, 2, 1024), dtype=f8)
        lsv = (ls + 6.0).astype(f8).reshape(K, 2, 1024)
        epv = eps.astype(f8).reshape(K, 2, 1024)
        le[:, :, 0, :] = lsv
        le[:, :, 1, :] = epv
        return np.ascontiguousarray(le)

    lsT_all = inputs["ls_w"].transpose(0, 2, 1)
    epsT_all = inputs["eps_w"].transpose(0, 2, 1)
    shared = {
        "muT0": np.ascontiguousarray(inputs["mu_w0"].astype(f16).T),
        "leT0": le_tile(inputs["ls_w0"].T, inputs["eps_w0"].T),
        "muT": np.ascontiguousarray(inputs["mu_w"].astype(f16).transpose(0, 2, 1)),
        "leT": np.stack(
            [le_tile(lsT_all[i], epsT_all[i]) for i in range(NL)], axis=0
        ),
        "muhT": head_tile(inputs["mu_hw"]),
        "lshT": head_tile(inputs["ls_hw"]),
        "epshT": head_tile(inputs["eps_hw"]),
        "mub": bias_tile(inputs["mu_b0"], inputs["mu_b"]),
        "lsb": bias_tile(inputs["ls_b0"], inputs["ls_b"]),
        "epsb": bias_tile(inputs["eps_b0"], inputs["eps_b"]),
        "muhb": inputs["mu_hb"].reshape(1, TO).astype(np.float32),
        "lshb": inputs["ls_hb"].reshape(1, TO).astype(np.float32),
        "epshb": inputs["eps_hb"].reshape(1, TO).astype(np.float32),
    }
    xT = inputs["x"].astype(f16).T  # [IN, B]
    task = inputs["task"].astype(np.float32)
    in_maps = []
    for c in range(NCORES):
        m = dict(shared)
        xc = xT[:, c * BC:(c + 1) * BC]  # [IN, BC]
        m["xT"] = np.ascontiguousarray(xc.reshape(KT_IN, 128, BC).transpose(1, 0, 2))
        m["taskf"] = np.ascontiguousarray(
            task[c * BC:(c + 1) * BC].reshape(BC // 128, 128).T
        )
        in_maps.append(m)
    return in_maps


def kernel(**inputs):
    inputs = {k: np.asarray(v) for k, v in inputs.items()}
    if "nc" not in _CACHE:
        _CACHE["nc"] = build_nc()
    nc = _CACHE["nc"]
    in_maps = _prep_host(inputs)
    res = run_bass_kernel_spmd(nc, in_maps, core_ids=list(range(NCORES)))
    out = np.concatenate([res.results[c]["out"] for c in range(NCORES)], axis=0)
    return out.astype(np.float32)


if __name__ == "__main__":
    nc = build_nc()
    print("built ok")


# revision 32
# speedup vs baseline: 1.0073x; 1.0073x over previous
"""Bayesian multi-task MLP (moe_routing) — Trainium2 Bass/Tile kernel.

Reference computation (per forward):
    w   = mu + exp(ls) * eps                    (Bayesian reparameterization)
    h   = relu(x @ w0.T + b0)                   [4096, 2048]
    h   = relu(h @ wi.T + bi)   for i in 0..2   (2048 -> 2048)
    out = (h @ hw[t].T + hb[t]) routed per-sample by task id   [4096, 10]

Distribution: pure data-parallel over the batch — each of the 8 cores gets
512 rows of x/task and a full replica of the (device-prepared) Bayesian
params.  No collectives.

Host-side prep is layout/dtype only:
  * weight factors transposed to [in, out] so the PE consumes them without
    on-chip transposes (activations stay transposed [hid, batch] throughout);
  * mu / x cast to fp16; ls shipped as (ls + 6) in fp8e4m3 (centered near 0
    where fp8 has precision); eps shipped as fp8e4m3; ls and eps interleaved
    row-wise into one tensor so each DMA line is 2 KB.

On-device weight prep per 4-k-tile group:
  ACT   t = exp(ls' - 6)        one op per group, bias AP folds the shift
  DVE   n = t * eps8            tensor_tensor, 1x (fp8 operand)
  DVE   w = n + mu              all-fp16 flat-view tensor_tensor -> 2x mode
  PE    8 accumulating matmuls  (stationary = weight tile, moving = hT batch)

Trace-driven design notes (measured on the 8-core SPMD run):
  * In steady state the PE issues 512-row matmuls back-to-back at the full
    2.4 GHz warm clock (216 ns inter-start, p25=p75); the kernel is at the
    fp16 PE roofline there.  Remaining span = ~197us of matmul issues
    + ~14us fixed startup (framework preamble + first weight chain)
    + ~5us tail (routing select + output DMA + final barrier).
  * scalar_tensor_tensor always runs the DVE at 1 elem/lane/cycle; the
    mult(1x fp8) + flat-view add(2x) pair hits the same 1.5 cyc/elem floor
    with one fewer op per tile.  3D strided views defeat the DVE 2x mode —
    fp16 operands must be flat 1D-free APs.
  * A dummy 1-elem activation precedes all DMA issues so the exp-LUT
    ACT_TABLE_LOAD's table fetch doesn't queue behind weight DMAs (-5us).
  * x DMAs ride the GpSimd DGE queue in parallel with the Sync queue's
    weight-stream issues.  (Routing the big eps streams there regressed
    30us — the GpSimd DGE handles large transfers poorly; weight DMAs
    stay on Sync.)
  * 4 head psum chains share one PSUM bank; one masked-mult + reduce +
    one output DMA finish the routing select in ~5us.
  * Tried and rejected: GPSIMD fp8->fp16 eps upconvert (CAST measures
    ~3.9 cyc/elem there), KG=8 groups (starves the 2-buf pipeline),
    mixed fp16/fp8 eps (+7MB DMA, 3 issues/group saturate the Sync DGE),
    deferred relu emission, deeper DMA lookahead (all within-noise worse).
"""

import numpy as np

import concourse.bacc as bacc
import concourse.mybir as mybir
from concourse.bass_utils import run_bass_kernel_spmd
from concourse.tile import TileContext

NCORES = 8
B, IN, H, OUT, T, NL = 4096, 1024, 2048, 10, 10, 3
BC = B // NCORES           # batch rows per core = 512
TO = T * OUT               # flattened head outputs = 100
KT_IN = IN // 128          # k-tiles in layer 0 = 8
KT_H = H // 128            # k-tiles in hidden layers = 16
KG = 4                     # k-tiles per grouped DMA / elementwise op
OT_HALF = 8                # out-tiles per half (8 PSUM banks)

F16 = mybir.dt.float16
F32 = mybir.dt.float32
F8 = mybir.dt.float8e4
ALU = mybir.AluOpType
ACTF = mybir.ActivationFunctionType


def build_nc():
    # Bacc (not raw Bass): its compile() pass legalizes multi-wait sync_infos
    # into EventSemaphore instructions (TRN2 allows 1 wait per instruction).
    nc = bacc.Bacc(trn_type="TRN2")

    # ---- per-core DRAM I/O ----
    xT = nc.dram_tensor("xT", [128, KT_IN, BC], F16, kind="ExternalInput")
    muT0 = nc.dram_tensor("muT0", [IN, H], F16, kind="ExternalInput")
    # ls/eps interleaved: [k, half, {ls,eps}, 1024] -> 2KB contiguous lines
    leT0 = nc.dram_tensor("leT0", [IN, 2, 2, 1024], F8, kind="ExternalInput")
    muT = nc.dram_tensor("muT", [NL, H, H], F16, kind="ExternalInput")
    leT = nc.dram_tensor("leT", [NL, H, 2, 2, 1024], F8, kind="ExternalInput")
    # head weights pre-tiled on host to [128, k, to] (contiguous per partition)
    muhT = nc.dram_tensor("muhT", [128, KT_H, TO], F16, kind="ExternalInput")
    lshT = nc.dram_tensor("lshT", [128, KT_H, TO], F16, kind="ExternalInput")
    epshT = nc.dram_tensor("epshT", [128, KT_H, TO], F16, kind="ExternalInput")
    # biases for the 4 dense layers, pre-tiled [128, layer, otile], fp32
    mub = nc.dram_tensor("mub", [128, NL + 1, KT_H], F32, kind="ExternalInput")
    lsb = nc.dram_tensor("lsb", [128, NL + 1, KT_H], F32, kind="ExternalInput")
    epsb = nc.dram_tensor("epsb", [128, NL + 1, KT_H], F32, kind="ExternalInput")
    muhb = nc.dram_tensor("muhb", [1, TO], F32, kind="ExternalInput")
    lshb = nc.dram_tensor("lshb", [1, TO], F32, kind="ExternalInput")
    epshb = nc.dram_tensor("epshb", [1, TO], F32, kind="ExternalInput")
    taskf = nc.dram_tensor("taskf", [128, BC // 128], F32, kind="ExternalInput")
    out = nc.dram_tensor("out", [BC, OUT], F32, kind="ExternalOutput")

    with TileContext(nc) as tc:
        with (
            tc.tile_pool(name="const", bufs=1) as cpool,
            tc.tile_pool(name="wstream", bufs=4) as wpool,
            tc.tile_pool(name="hbuf", bufs=3) as hpool,
            tc.tile_pool(name="sel", bufs=2) as spool,
            tc.tile_pool(name="psum", bufs=8, space="PSUM") as ppool,
        ):
            # per-partition -6.0 bias: folds the host's +6 fp8 shift back out
            # inside the exp, so the noise multiply needs no scalar factor
            neg6 = cpool.tile([128, 1], F32, name="neg6")
            nc.vector.memset(neg6, -6.0)
            # dummy activation before any DMA is issued: the compiler inserts
            # ACT_TABLE_LOAD before the first Scalar activation, and its table
            # fetch must not queue behind the big weight DMAs (~5us startup)
            warm = cpool.tile([1, 1], F32, name="warm")
            warm2 = cpool.tile([1, 1], F32, name="warm2")
            nc.vector.memset(warm, 0.0)
            nc.scalar.activation(out=warm2, in_=warm, func=ACTF.Exp)
            # HAM warm-up: ~12 dummy matmuls on memset data, gated only by a
            # DVE memset, run cold at ~0.43us each starting ~6us in and end
            # before the first real matmul's data lands (~15us).  The HAM
            # clock gate needs ~3.4us of sustained PE activity to release
            # 2.4 GHz; without this the first real ~16 matmuls run at 1.2 GHz.
            wmm = cpool.tile([128, 640], F16, name="wmm")
            nc.vector.memset(wmm, 0.0)
            wps = ppool.tile([128, BC], F32, tag="mm", name="wps")
            for i in range(12):
                nc.tensor.matmul(
                    wps, lhsT=wmm[:, 0:128], rhs=wmm[:, 128:640],
                    start=(i == 0), stop=(i == 11),
                )

            # ---- dense-layer biases b = mu + exp(ls)*eps  (tiny, fp32) ----
            # Built lazily so the first weight-group DMAs lead the DMA queue;
            # the bias is only consumed at the first relu, ~50us in.
            _bias_cache = []

            def get_bias():
                if _bias_cache:
                    return _bias_cache[0]
                bias_mu = cpool.tile([128, NL + 1, KT_H], F32, name="bias_mu")
                bias_ls = cpool.tile([128, NL + 1, KT_H], F32, name="bias_ls")
                bias_eps = cpool.tile([128, NL + 1, KT_H], F32, name="bias_eps")
                nc.sync.dma_start(out=bias_mu, in_=mub.ap())
                nc.sync.dma_start(out=bias_ls, in_=lsb.ap())
                nc.sync.dma_start(out=bias_eps, in_=epsb.ap())
                bias = cpool.tile([128, NL + 1, KT_H], F32, name="bias")
                nc.scalar.activation(out=bias, in_=bias_ls, func=ACTF.Exp)
                nc.vector.tensor_mul(bias, bias, bias_eps)
                nc.vector.tensor_add(bias, bias, bias_mu)
                _bias_cache.append(bias)
                return bias

            # ---- layer 0 input: xT resident in SBUF (graduated chunks so the
            # first matmul only waits on a 128KB slice; chunk 0 is issued
            # first, the rest after the first weight group's DMAs) ----
            # x DMAs go through the GpSimd DGE queue (~25ns issue) so they
            # never contend with the weight-stream issues on the Sync queue
            hT_x = hpool.tile([128, KT_IN, BC], F16, tag="hT")
            nc.gpsimd.dma_start(out=hT_x[:, 0:1, :], in_=xT.ap()[:, 0:1, :])

            _x_sched = {1: (1, 1), 2: (2, 2), 3: (4, 4)}

            def issue_x_chunk(gi):
                if gi in _x_sched:
                    xk, xc = _x_sched[gi]
                    nc.gpsimd.dma_start(
                        out=hT_x[:, xk:xk + xc, :], in_=xT.ap()[:, xk:xk + xc, :]
                    )

            def ff_layer(hT_in, kt, mu_ap, le_ap, bias_l, first=False,
                         fill=False, after_first_group=None):
                """hT_out[out, b] = relu(w @ hT_in + b); w = mu + exp(ls)*eps.

                mu_ap: [kt*128, H] fp16 transposed view.
                le_ap: [kt*128, 2, 2, 1024] fp8 view; [., h, 0, :] is ls+6 for
                column half h, [., h, 1, :] is eps.
                """
                hT_out = hpool.tile([128, KT_H, BC], F16, tag="hT", name="hT_out")
                for half in range(2):
                    psums = []
                    for o8 in range(OT_HALF):
                        ps = ppool.tile([128, BC], F32, tag="mm", name="ps")
                        psums.append(ps)
                    ocols = slice(half * 1024, (half + 1) * 1024)
                    # The weight stream runs at ~290GB/s against ~350GB/s DMA
                    # capacity, so the prefetch buffer only fills slowly; for
                    # the first ~60us (L0 + first half of L1) use small groups
                    # to keep per-group transfer latency off the PE's critical
                    # path.  Steady state uses KG-sized groups.
                    if first and half == 0:
                        gsizes = [1, 1, 2]
                        while sum(gsizes) < kt:
                            gsizes.append(min(KG, kt - sum(gsizes)))
                    else:
                        gsizes = [KG] * (kt // KG)
                    k0 = 0
                    for gi, gs in enumerate(gsizes):
                        grows = slice(k0 * 128, (k0 + gs) * 128)
                        mu_g = wpool.tile([128, KG, 1024], F16, tag="mu", name="mu_g")
                        le_g = wpool.tile([128, KG, 2048], F8, tag="le", name="le_g")
                        if first or fill:
                            # fill region: split the group's ls and eps into
                            # two sub-DMAs — the exp only waits on the ls
                            # half, halving the prep chain's DMA latency
                            # while the prefetch buffer is still filling
                            nc.sync.dma_start(
                                out=le_g[:, :gs, 0:1024],
                                in_=le_ap[grows, half, 0].rearrange(
                                    "(g p) o -> p g o", p=128
                                ),
                            )
                            nc.sync.dma_start(
                                out=le_g[:, :gs, 1024:2048],
                                in_=le_ap[grows, half, 1].rearrange(
                                    "(g p) o -> p g o", p=128
                                ),
                            )
                        else:
                            nc.sync.dma_start(
                                out=le_g[:, :gs, :],
                                in_=le_ap[grows, half].rearrange(
                                    "(g p) t o -> p g (t o)", p=128
                                ),
                            )
                        nc.sync.dma_start(
                            out=mu_g[:, :gs, :],
                            in_=mu_ap[grows, ocols].rearrange(
                                "(g p) o -> p g o", p=128
                            ),
                        )
                        if first and half == 0 and after_first_group:
                            after_first_group(gi)
                        # group elementwise: one op covers gs k-tiles.  All
                        # fp16 operands use flat 1D-free views ([128, gs*1024])
                        # so the DVE 2x perf mode engages on the add.
                        t_g = wpool.tile([128, KG, 1024], F16, tag="t", name="t_g")
                        w_g = wpool.tile([128, KG, 1024], F16, tag="w", name="w_g")
                        t_fl = t_g.rearrange("p g o -> p (g o)")[:, :gs * 1024]
                        w_fl = w_g.rearrange("p g o -> p (g o)")[:, :gs * 1024]
                        mu_fl = mu_g.rearrange("p g o -> p (g o)")[:, :gs * 1024]
                        nc.scalar.activation(
                            out=t_g[:, :gs, :], in_=le_g[:, :gs, 0:1024],
                            func=ACTF.Exp, bias=neg6,
                        )
                        nc.vector.tensor_tensor(
                            t_g[:, :gs, :], t_g[:, :gs, :],
                            le_g[:, :gs, 1024:2048], ALU.mult,
                        )
                        nc.vector.tensor_add(w_fl, t_fl, mu_fl)
                        for ks in range(gs):
                            k = k0 + ks
                            for o8 in range(OT_HALF):
                                nc.tensor.matmul(
                                    psums[o8],
                                    lhsT=w_g[:, ks, o8 * 128:(o8 + 1) * 128],
                                    rhs=hT_in[:, k, :],
                                    start=(k == 0),
                                    stop=(k == kt - 1),
                                )
                        k0 += gs
                    for o8 in range(OT_HALF):
                        o = half * OT_HALF + o8
                        nc.scalar.activation(
                            out=hT_out[:, o, :],
                            in_=psums[o8],
                            func=ACTF.Relu,
                            bias=get_bias()[:, bias_l, o:o + 1],
                        )
                return hT_out

            cur = ff_layer(hT_x, KT_IN, muT0.ap(), leT0.ap(), 0, first=True,
                           after_first_group=issue_x_chunk)
            for l in range(NL - 1):
                cur = ff_layer(cur, KT_H, muT.ap()[l], leT.ap()[l], l + 1,
                               fill=(l == 0))

            # ---- tail constants (issued before the last layer so their DMAs
            # land while its GEMMs run) ----
            hb_mu = cpool.tile([1, TO], F32)
            hb_ls = cpool.tile([1, TO], F32)
            hb_eps = cpool.tile([1, TO], F32)
            nc.sync.dma_start(out=hb_mu, in_=muhb.ap())
            nc.sync.dma_start(out=hb_ls, in_=lshb.ap())
            nc.sync.dma_start(out=hb_eps, in_=epshb.ap())
            hb_f = cpool.tile([1, TO], F32)
            nc.scalar.activation(out=hb_f, in_=hb_ls, func=ACTF.Exp)
            nc.vector.tensor_mul(hb_f, hb_f, hb_eps)
            nc.vector.tensor_add(hb_f, hb_f, hb_mu)
            hb16 = cpool.tile([1, TO], F16)
            nc.vector.tensor_copy(out=hb16, in_=hb_f)
            ones1 = cpool.tile([1, 128], F16)
            nc.vector.memset(ones1, 1.0)

            # head weights: whT[p, k, to] = mu + exp(ls)*eps  (fp16, ~1.2MB)
            wh_mu = cpool.tile([128, KT_H, TO], F16)
            wh_ls = cpool.tile([128, KT_H, TO], F16)
            wh_eps = cpool.tile([128, KT_H, TO], F16)
            nc.sync.dma_start(out=wh_mu, in_=muhT.ap())
            nc.sync.dma_start(out=wh_ls, in_=lshT.ap())
            nc.sync.dma_start(out=wh_eps, in_=epshT.ap())
            whT = cpool.tile([128, KT_H, TO], F16)
            nc.scalar.activation(out=whT, in_=wh_ls, func=ACTF.Exp)
            nc.vector.tensor_mul(whT, whT, wh_eps)
            nc.vector.tensor_add(whT, whT, wh_mu)

            taskt = cpool.tile([128, BC // 128], F32)
            nc.sync.dma_start(out=taskt, in_=taskf.ap())
            iota10 = cpool.tile([128, T], mybir.dt.int32)
            nc.gpsimd.iota(iota10, [[1, T]], base=0, channel_multiplier=0)
            iota10f = cpool.tile([128, T], F32)
            nc.vector.tensor_copy(out=iota10f, in_=iota10)
            # one-hot rows for all 4 m-tiles, built before the last layer
            MT = BC // 128
            onehot4 = cpool.tile([128, MT, T], F32, name="onehot4")
            for m in range(MT):
                nc.vector.tensor_single_scalar(
                    out=onehot4[:, m, :], in_=iota10f,
                    scalar=taskt[:, m:m + 1], op=ALU.is_equal,
                )

            # ---- last hidden layer ----
            cur = ff_layer(cur, KT_H, muT.ap()[NL - 1], leT.ap()[NL - 1], NL)

            # ---- heads + routing select: all 4 m-tiles in one PSUM bank ----
            ps = ppool.tile([128, MT * TO], F32, tag="mm", name="ps_head")
            for m in range(MT):
                cols = slice(m * TO, (m + 1) * TO)
                for k in range(KT_H):
                    nc.tensor.matmul(
                        ps[:, cols],
                        lhsT=cur[:, k, m * 128:(m + 1) * 128],
                        rhs=whT[:, k, :],
                        start=(k == 0),
                        stop=False,
                    )
                nc.tensor.matmul(
                    ps[:, cols], lhsT=ones1[:1, :], rhs=hb16[:1, :],
                    start=False, stop=True,
                )
            masked = spool.tile([128, MT, OUT, T], F32, name="masked")
            ps_v = ps.rearrange("p (m t o) -> p m o t", m=MT, t=T)
            oh_v = onehot4.unsqueeze(2).broadcast_to([128, MT, OUT, T])
            nc.vector.tensor_tensor(masked, ps_v, oh_v, ALU.mult)
            outm = spool.tile([128, MT, OUT], F32, name="outm")
            nc.vector.tensor_reduce(
                out=outm, in_=masked, axis=mybir.AxisListType.X, op=ALU.add
            )
            nc.sync.dma_start(
                out=out.ap().rearrange("(m p) o -> p m o", p=128), in_=outm
            )

    nc.finalize()
    return nc


_CACHE = {}


def _prep_host(inputs):
    """Layout/dtype prep + batch sharding. Returns list of per-core in_maps."""
    import ml_dtypes

    f16 = np.float16
    f8 = ml_dtypes.float8_e4m3fn

    def bias_tile(b0, b):  # [4, H] -> [128, 4, 16]
        arr = np.concatenate([b0[None], b], 0).astype(np.float32)
        return np.ascontiguousarray(arr.reshape(NL + 1, KT_H, 128).transpose(2, 0, 1))

    def head_tile(a):  # [T, OUT, H] -> headT [H, TO] -> [128, 16, TO]
        aT = a.reshape(TO, H).astype(f16).T
        return np.ascontiguousarray(aT.reshape(KT_H, 128, TO).transpose(1, 0, 2))

    def le_tile(ls, eps):
        # [K, H] x2 -> [K, half, {ls,eps}, 1024] fp8, ls pre-shifted by +6
        K = ls.shape[0]
        le = np.empty((K, 2, 2, 1024), dtype=f8)
        lsv = (ls + 6.0).astype(f8).reshape(K, 2, 1024)
        epv = eps.astype(f8).reshape(K, 2, 1024)
        le[:, :, 0, :] = lsv
        le[:, :, 1, :] = epv
        return np.ascontiguousarray(le)

    lsT_all = inputs["ls_w"].transpose(0, 2, 1)
    epsT_all = inputs["eps_w"].transpose(0, 2, 1)
    shared = {
        "muT0": np.ascontiguousarray(inputs["mu_w0"].astype(f16).T),
        "leT0": le_tile(inputs["ls_w0"].T, inputs["eps_w0"].T),
        "muT": np.ascontiguousarray(inputs["mu_w"].astype(f16).transpose(0, 2, 1)),
        "leT": np.stack(
            [le_tile(lsT_all[i], epsT_all[i]) for i in range(NL)], axis=0
        ),
        "muhT": head_tile(inputs["mu_hw"]),
        "lshT": head_tile(inputs["ls_hw"]),
        "epshT": head_tile(inputs["eps_hw"]),
        "mub": bias_tile(inputs["mu_b0"], inputs["mu_b"]),
        "lsb": bias_tile(inputs["ls_b0"], inputs["ls_b"]),
        "epsb": bias_tile(inputs["eps_b0"], inputs["eps_b"]),
        "muhb": inputs["mu_hb"].reshape(1, TO).astype(np.float32),
        "lshb": inputs["ls_hb"].reshape(1, TO).astype(np.float32),
        "epshb": inputs["eps_hb"].reshape(1, TO).astype(np.float32),
    }
    xT = inputs["x"].astype(f16).T  # [IN, B]
    task = inputs["task"].astype(np.float32)
    in_maps = []
    for c in range(NCORES):
        m = dict(shared)
        xc = xT[:, c * BC:(c + 1) * BC]  # [IN, BC]
        m["xT"] = np.ascontiguousarray(xc.reshape(KT_IN, 128, BC).transpose(1, 0, 2))
        m["taskf"] = np.ascontiguousarray(
            task[c * BC:(c + 1) * BC].reshape(BC // 128, 128).T
        )
        in_maps.append(m)
    return in_maps


def kernel(**inputs):
    inputs = {k: np.asarray(v) for k, v in inputs.items()}
    if "nc" not in _CACHE:
        _CACHE["nc"] = build_nc()
    nc = _CACHE["nc"]
    in_maps = _prep_host(inputs)
    res = run_bass_kernel_spmd(nc, in_maps, core_ids=list(range(NCORES)))
    out = np.concatenate([res.results[c]["out"] for c in range(NCORES)], axis=0)
    return out.astype(np.float32)


if __name__ == "__main__":
    nc = build_nc()
    print("built ok")
